# revision 1
# baseline (speedup 1.0000x reference)
"""DeepSeek-style MHA (GQA + neox RoPE + causal) on 8 TRN2 NeuronCores.

Sharding: data-parallel over batch (2) x tensor-parallel over heads (4).
Core c handles batch b = c//4 and q-heads [4g..4g+3], kv-head g, g = c%4.
Each core computes its 4 heads' attention and a partial o_proj
(rows 512g..512g+512 of w_o); the host sums the 4 partials per batch
(the "all-reduce after o_proj" of the row-parallel sharding).

On-device layout is transposed ([dim, token]) throughout so that no
activation transpose is needed after the initial X^T:
  - QKV computed as qkv^T = matmul(lhsT=Wqkv, rhs=X^T)
  - scores^T[k,q] = matmul(lhsT=kT, rhs=qT); softmax runs along the
    partition (k) axis: exp on ScalarE, denominator via an all-ones
    stationary matmul, PV as matmul(lhsT=v_natural, rhs=probs^T).
  - o_proj: Y[t,c] = matmul(lhsT=attnT, rhs=Wo) gives natural layout.
Matmuls run in float32r (full-rate fp32 on the PE at free-dim>=256);
only the probs/V path is bf16.
"""

import sys

if '/opt/trn_rl_repo' not in sys.path:
    sys.path.insert(0, '/opt/trn_rl_repo')

import numpy as np
from contextlib import ExitStack

B, S, HID = 2, 2048, 2048
NUM_HEADS, NUM_KV_HEADS, D = 16, 4, 128
Q_SIZE = NUM_HEADS * D
KV_SIZE = NUM_KV_HEADS * D
ROPE_THETA = 10000.0
TP = 4                      # head-parallel degree (heads per core = 4)
HPC = NUM_HEADS // TP       # q heads per core = 4
CH = 256                    # token chunk for qkv projection
NCH = S // CH
QC = 512                    # query chunk for attention
NQC = S // QC
NKT = S // D                # key tiles
NM = HPC + 2                # qkv M-tiles per core: 4 q heads + k + v
SCALE = float(1.0 / np.sqrt(D))

_prog = None


def _build_program():
    import os
    stages = int(os.environ.get("BASS_STAGES", "3"))
    from concourse import bacc, mybir, tile

    F32R = mybir.dt.float32r
    F32 = mybir.dt.float32
    BF16 = mybir.dt.bfloat16
    AF = mybir.ActivationFunctionType
    ALU = mybir.AluOpType

    nc = bacc.Bacc("TRN2", target_bir_lowering=False, debug=False)
    x_d = nc.dram_tensor("x", [S, HID], F32, kind="ExternalInput").ap()
    wqkv_d = nc.dram_tensor("wqkv", [HID, NM * D], F32, kind="ExternalInput").ap()
    wo_d = nc.dram_tensor("wo", [HPC * D, HID], F32, kind="ExternalInput").ap()
    cs2_d = nc.dram_tensor("cs2", [D, S], F32, kind="ExternalInput").ap()
    sn2_d = nc.dram_tensor("sn2", [D, S], F32, kind="ExternalInput").ap()
    mask_d = nc.dram_tensor("masks", [D, 4 * QC], BF16, kind="ExternalInput").ap()
    id_d = nc.dram_tensor("idin", [D, D], F32, kind="ExternalInput").ap()
    out_d = nc.dram_tensor("out", [S, HID], F32, kind="ExternalOutput").ap()

    with tile.TileContext(nc) as tc, ExitStack() as octx:
        pers = octx.enter_context(tc.tile_pool(name="pers", bufs=1))
        psp = octx.enter_context(tc.tile_pool(name="psp", bufs=8, space="PSUM"))

        def psum():
            return psp.tile([D, 512], F32, tag="ps", name="ps")

        id_f = pers.tile([D, D], F32R, tag="idf")
        nc.sync.dma_start(id_f[:], id_d.bitcast(F32R))
        ident_r = id_f[:]
        ones_bf = pers.tile([D, D], BF16, tag="ones")
        nc.vector.memset(ones_bf[:], 1.0)
        cs2 = pers.tile([D, S], F32, tag="cs2")
        sn2 = pers.tile([D, S], F32, tag="sn2")
        maskt = pers.tile([D, 4 * QC], BF16, tag="maskt")

        qT = [pers.tile([D, S], F32R, tag=f"qT{h}", name=f"qT{h}") for h in range(HPC)]
        kT = pers.tile([D, S], F32R, tag="kT")
        v_nat = pers.tile([D, S], BF16, tag="vnat")  # slice kt -> v[kt*128:(kt+1)*128, :]

        # ---------------- Stage A: X^T + QKV projection + RoPE ----------------
        with ExitStack() as sA:
            wqp = sA.enter_context(tc.tile_pool(name="wqp", bufs=1))
            wqkv_t = [wqp.tile([D, NM * D], F32R, tag=f"wq{kt}", name=f"wq{kt}")
                      for kt in range(NKT)]
            xnp = sA.enter_context(tc.tile_pool(name="xnp", bufs=2))
            xtp = sA.enter_context(tc.tile_pool(name="xtp", bufs=2))
            rsp = sA.enter_context(tc.tile_pool(name="rsp", bufs=4))
            vtp = sA.enter_context(tc.tile_pool(name="vtp", bufs=2))

            xt_tiles = [None] * NCH

            def load_transpose(c):
                xn = []
                for r in range(2):
                    t = xnp.tile([D, HID], F32R, tag=f"xn{r}", name=f"xn{r}")
                    for q in range(4):
                        nc.sync.dma_start(
                            t[:, q * 512:(q + 1) * 512],
                            x_d[c * CH + r * D: c * CH + (r + 1) * D,
                                q * 512:(q + 1) * 512].bitcast(F32R))
                    xn.append(t)
                xts = xtp.tile([D, NKT * CH], F32R, tag="xt", name="xt")
                xt_tiles[c] = xts
                for ht in range(0, NKT, 2):
                    tp = psum()
                    for hh in range(2):
                        for r in range(2):
                            nc.tensor.transpose(
                                tp[:, hh * CH + r * D: hh * CH + (r + 1) * D].bitcast(F32R),
                                xn[r][:, (ht + hh) * D:(ht + hh + 1) * D], ident_r)
                    nc.scalar.copy(xts[:, ht * CH:(ht + 2) * CH], tp[:])

            def qkv_chunk(c):
                xts = xt_tiles[c]
                cols = slice(c * CH, (c + 1) * CH)
                for m in range(NM):
                    qp = psum()
                    for kt in range(NKT):
                        nc.tensor.matmul(
                            qp[:, 0:CH],
                            wqkv_t[kt][:, m * D:(m + 1) * D],
                            xts[:, kt * CH:(kt + 1) * CH],
                            start=(kt == 0), stop=(kt == NKT - 1))
                    if m < HPC + 1:
                        # rope: out[0:64] = x0*cos - x1*sin ; out[64:128] = x1*cos + x0*sin
                        # x_sw = [x1; x0] via partition-swapping DMA; sn2 carries
                        # [-sin; +sin] so out = x*cs2 + x_sw*sn2, all partition-aligned.
                        dest = qT[m] if m < HPC else kT
                        e_t = rsp.tile([D, CH], F32, tag="ev", name="ev")
                        nc.vector.tensor_copy(e_t[:], qp[:, 0:CH])
                        x_sw = rsp.tile([D, CH], F32, tag="xsw", name="xsw")
                        nc.sync.dma_start(x_sw[0:64, :], e_t[64:128, :])
                        nc.sync.dma_start(x_sw[64:128, :], e_t[0:64, :])
                        a_t = rsp.tile([D, CH], F32, tag="ra", name="ra")
                        b_t = rsp.tile([D, CH], F32, tag="rb", name="rb")
                        nc.vector.tensor_tensor(a_t[:], e_t[:], cs2[:, cols], ALU.mult)
                        nc.vector.tensor_tensor(b_t[:], x_sw[:], sn2[:, cols], ALU.mult)
                        nc.vector.tensor_tensor(dest[:, cols], a_t[:], b_t[:], ALU.add)
                    else:
                        vt = vtp.tile([D, CH], F32R, tag="vt", name="vt")
                        nc.scalar.copy(vt[:], qp[:, 0:CH])
                        tp2 = psum()
                        for r in range(2):
                            nc.tensor.transpose(
                                tp2[:, r * D:(r + 1) * D].bitcast(F32R),
                                vt[:, r * D:(r + 1) * D], ident_r)
                        nc.scalar.copy(v_nat[:, 2 * c * D:(2 * c + 2) * D], tp2[:, 0:CH])

            # x chunk 0 first so the PE can start transposing immediately;
            # weights and rope tables stream in behind it.
            load_transpose(0)
            for kt in range(NKT):
                nc.sync.dma_start(
                    wqkv_t[kt][:], wqkv_d[kt * D:(kt + 1) * D, :].bitcast(F32R))
            nc.sync.dma_start(cs2[:], cs2_d)
            nc.sync.dma_start(sn2[:], sn2_d)
            nc.sync.dma_start(maskt[:], mask_d)
            for c in range(NCH):
                if c + 1 < NCH:
                    load_transpose(c + 1)
                qkv_chunk(c)

        if stages < 2:
            dbg = octx.enter_context(tc.tile_pool(name="dbg", bufs=2))
            for sdx, src in enumerate([qT[0], kT]):
                for ncx in range(4):
                    dt_ = dbg.tile([D, 512], F32, name="dt", tag="dt")
                    nc.vector.tensor_copy(dt_[:], src[:, ncx * 512:(ncx + 1) * 512].bitcast(F32))
                    nc.sync.dma_start(out_d[sdx * D:(sdx + 1) * D, ncx * 512:(ncx + 1) * 512], dt_[:])

        # ---------------- Stage B: attention ----------------
        if stages >= 2:
            wop = octx.enter_context(tc.tile_pool(name="wop", bufs=1))
            wo_s = wop.tile([D, HPC * HID], F32R, tag="wo")
            for h in range(HPC):
                nc.sync.dma_start(
                    wo_s[:, h * HID:(h + 1) * HID], wo_d[h * D:(h + 1) * D, :].bitcast(F32R))
            atp = octx.enter_context(tc.tile_pool(name="atp", bufs=1))
            attnT = [atp.tile([D, S], F32R, tag=f"at{h}", name=f"at{h}") for h in range(HPC)]
            ptp = octx.enter_context(tc.tile_pool(name="ptp", bufs=4))
            nrp = octx.enter_context(tc.tile_pool(name="nrp", bufs=4))

            for h in range(HPC):
                for qc in range(NQC):
                    nblk = 4 * qc + 4
                    qsl = slice(qc * QC, (qc + 1) * QC)
                    pvp = psum()
                    dnp = psum()

                    def emit_score(kt):
                        sp = psum()
                        nc.tensor.matmul(
                            sp[:], kT[:, kt * D:(kt + 1) * D], qT[h][:, qsl],
                            start=True, stop=True)
                        return sp

                    prev = emit_score(0)
                    for kt in range(nblk):
                        nxt = emit_score(kt + 1) if kt + 1 < nblk else None
                        pt_t = ptp.tile([D, QC], BF16, tag="pt", name="pt")
                        nc.scalar.activation(pt_t[:], prev[:], AF.Exp, scale=SCALE)
                        if kt >= 4 * qc:
                            msl = slice((kt - 4 * qc) * QC, (kt - 4 * qc + 1) * QC)
                            nc.vector.tensor_tensor(pt_t[:], pt_t[:], maskt[:, msl], ALU.mult)
                        nc.tensor.matmul(
                            pvp[:], v_nat[:, kt * D:(kt + 1) * D], pt_t[:],
                            start=(kt == 0), stop=(kt == nblk - 1))
                        nc.tensor.matmul(
                            dnp[:], ones_bf[:], pt_t[:],
                            start=(kt == 0), stop=(kt == nblk - 1))
                        prev = nxt
                    rc = nrp.tile([D, QC], F32, tag="rc", name="rc")
                    nc.vector.reciprocal(rc[:], dnp[:])
                    nc.vector.tensor_tensor(attnT[h][:, qsl], pvp[:], rc[:], ALU.mult)

        if stages == 2:
            dbg = octx.enter_context(tc.tile_pool(name="dbg", bufs=2))
            for sdx in range(HPC):
                for ncx in range(4):
                    dt_ = dbg.tile([D, 512], F32, name="dt", tag="dt")
                    nc.vector.tensor_copy(
                        dt_[:], attnT[sdx][:, ncx * 512:(ncx + 1) * 512].bitcast(F32))
                    nc.sync.dma_start(
                        out_d[sdx * D:(sdx + 1) * D, ncx * 512:(ncx + 1) * 512], dt_[:])

        # ---------------- Stage C: partial o_proj ----------------
        if stages >= 3:
            yvp = octx.enter_context(tc.tile_pool(name="yvp", bufs=4))
            for tt in range(S // D):
                for ncx in range(HID // 512):
                    yp = psum()
                    for h in range(HPC):
                        nc.tensor.matmul(
                            yp[:], attnT[h][:, tt * D:(tt + 1) * D],
                            wo_s[:, h * HID + ncx * 512: h * HID + (ncx + 1) * 512],
                            start=(h == 0), stop=(h == HPC - 1))
                    yt = yvp.tile([D, 512], F32, tag="yt", name="yt")
                    if (tt + ncx) % 2 == 0:
                        nc.scalar.copy(yt[:], yp[:])
                    else:
                        nc.vector.tensor_copy(yt[:], yp[:])
                    nc.sync.dma_start(
                        out_d[tt * D:(tt + 1) * D, ncx * 512:(ncx + 1) * 512], yt[:])

    nc.compile()
    return nc


def _get_program():
    global _prog
    if _prog is None:
        _prog = _build_program()
    return _prog


def _host_tables(positions_b):
    inv_freq = (1.0 / (ROPE_THETA ** (np.arange(0, D, 2, dtype=np.float32) / D))).astype(np.float32)
    ang = positions_b.astype(np.float32)[:, None] * inv_freq[None, :]   # [S, 64]
    cosT = np.cos(ang).T.astype(np.float32)                              # [64, S]
    sinT = np.sin(ang).T.astype(np.float32)
    cs2 = np.concatenate([cosT, cosT], axis=0)                           # [128, S]
    sn2 = np.concatenate([-sinT, sinT], axis=0)                          # signed for the add
    return np.ascontiguousarray(cs2), np.ascontiguousarray(sn2)


def _host_masks():
    import ml_dtypes
    k = np.arange(D)[:, None]
    j = np.arange(QC)[None, :]
    pats = [((m * D + k) <= j).astype(np.float32) for m in range(4)]
    masks = np.concatenate(pats, axis=1)                                 # [128, 4*512]
    return masks.astype(ml_dtypes.bfloat16)


def kernel(positions, hidden_states, w_qkv, w_o):
    from concourse.bass_utils import run_bass_kernel_spmd

    nc = _get_program()

    positions = np.asarray(positions)
    hidden_states = np.asarray(hidden_states, dtype=np.float32)
    w_qkv = np.asarray(w_qkv, dtype=np.float32)
    w_o = np.asarray(w_o, dtype=np.float32)

    masks = _host_masks()
    idin = np.eye(D, dtype=np.float32)
    tables = [_host_tables(positions[b]) for b in range(B)]

    in_maps = []
    for c in range(2 * TP):
        b, g = c // TP, c % TP
        wq_cols = np.concatenate([
            w_qkv[:, g * HPC * D:(g + 1) * HPC * D],          # 4 q heads
            w_qkv[:, Q_SIZE + g * D: Q_SIZE + (g + 1) * D],   # k head g
            w_qkv[:, Q_SIZE + KV_SIZE + g * D: Q_SIZE + KV_SIZE + (g + 1) * D],  # v head g
        ], axis=1)
        cs2, sn2 = tables[b]
        in_maps.append({
            "x": np.ascontiguousarray(hidden_states[b]),
            "wqkv": np.ascontiguousarray(wq_cols),
            "wo": np.ascontiguousarray(w_o[g * HPC * D:(g + 1) * HPC * D, :]),
            "cs2": cs2,
            "sn2": sn2,
            "masks": masks,
            "idin": idin,
        })

    res = run_bass_kernel_spmd(nc, in_maps, core_ids=list(range(2 * TP)))

    out = np.zeros((B, S, HID), dtype=np.float32)
    for c in range(2 * TP):
        b = c // TP
        out[b] += res.results[c]["out"]
    return out



# revision 14
# speedup vs baseline: 1.1777x; 1.1777x over previous
"""DeepSeek-style MHA (GQA + neox RoPE + causal) on 8 TRN2 NeuronCores.

Sharding: data-parallel over batch (2) x tensor-parallel over kv-heads (4).
Core c handles batch b = c//4, kv head g = c%4, q-heads [4g..4g+4). Each core
computes its 4 heads' attention and a partial o_proj (rows 512g..512g+512 of
w_o); the host sums the 4 partials per batch.

All heavy matmuls run as fp8e4 DoubleRow (2x128-deep contraction per
instruction at 0.5 cycles/row). Full precision is kept with two-plane
(hi/lo) fp8 splits computed on the host:
  x ~= xh + xl,  w ~= (wh + wl)/SCALE   (weights pre-scaled into e4m3 range)
  w.x ~= wh.xh + wl.xh + wh.xl          (three DoubleRow passes)
X arrives pre-transposed from the host ([hid, tok]); no PE transposes at all.
Scores run in bf16 (q/k from RoPE on the DVE). Softmax: exp on the Act engine
(bias -2 for fp8/bf16 range safety), denominator via an appended ones-column
(=64, cancelling the w_qkv quant scale) on the PV stationary so PV+denominator
share PSUM; normalization uses a gpsimd partition_broadcast of 1/denom.
o_proj: attn split hi/lo on the DVE, three DoubleRow passes against host-split
w_o planes; y lands in PSUM, is copied to bf16 and DMA'd out raw (x32 weight
scale removed on the host while summing partials).
"""

import sys

if '/opt/trn_rl_repo' not in sys.path:
    sys.path.insert(0, '/opt/trn_rl_repo')

import numpy as np
from contextlib import ExitStack

B, S, HID = 2, 2048, 2048
NUM_HEADS, NUM_KV_HEADS, D = 16, 4, 128
Q_SIZE = NUM_HEADS * D
KV_SIZE = NUM_KV_HEADS * D
ROPE_THETA = 10000.0
TP = 4                      # kv-head parallel degree
HPC = NUM_HEADS // TP       # q heads per core = 4
NKT = HID // D              # 16 hid tiles
QK_M = HPC + 1              # q0..q3 + k m-tiles
NCH = 4                     # token chunks of 512 for qkv
NQC = 4                     # query chunks of 512 for attention
SCALE = float(1.0 / np.sqrt(D))
WQ_SCALE = 64.0             # e4m3 range scaling for w_qkv (std 1/sqrt(2048))
WO_SCALE = 32.0             # e4m3 range scaling for w_o  (std 1/sqrt(512))

_prog = None


def _build_program():
    import os
    stages = int(os.environ.get("BASS_STAGES", "3"))
    from concourse import bacc, mybir, tile

    F32 = mybir.dt.float32
    BF16 = mybir.dt.bfloat16
    F8 = mybir.dt.float8e4
    AF = mybir.ActivationFunctionType
    ALU = mybir.AluOpType
    DR = mybir.MatmulPerfMode.DoubleRow

    nc = bacc.Bacc("TRN2", target_bir_lowering=False, debug=False)
    xh_d = nc.dram_tensor("xh", [D, NKT, S], F8, kind="ExternalInput").ap()
    xl_d = nc.dram_tensor("xl", [D, NKT, S], F8, kind="ExternalInput").ap()
    wqh_d = nc.dram_tensor("wqh", [D, NKT, QK_M * D], F8, kind="ExternalInput").ap()
    wql_d = nc.dram_tensor("wql", [D, NKT, QK_M * D], F8, kind="ExternalInput").ap()
    wvh_d = nc.dram_tensor("wvh", [D, NKT, D], F8, kind="ExternalInput").ap()
    wvl_d = nc.dram_tensor("wvl", [D, NKT, D], F8, kind="ExternalInput").ap()
    woh_d = nc.dram_tensor("woh", [D, HPC, HID], F8, kind="ExternalInput").ap()
    wol_d = nc.dram_tensor("wol", [D, HPC, HID], F8, kind="ExternalInput").ap()
    cs_d = nc.dram_tensor("cs2", [D, S], BF16, kind="ExternalInput").ap()
    sn_d = nc.dram_tensor("sn2", [D, S], BF16, kind="ExternalInput").ap()
    mask_d = nc.dram_tensor("masks", [D, 4 * 512], BF16, kind="ExternalInput").ap()
    out_d = nc.dram_tensor("out", [S, HID], BF16, kind="ExternalOutput").ap()

    with tile.TileContext(nc) as tc, ExitStack() as octx:
        pers = octx.enter_context(tc.tile_pool(name="pers", bufs=1))

        xh_s = pers.tile([D, NKT, S], F8, tag="xh", name="xh")
        xl_s = pers.tile([D, NKT, S], F8, tag="xl", name="xl")
        wqh_s = pers.tile([D, NKT, QK_M * D], F8, tag="wqh", name="wqh")
        wql_s = pers.tile([D, NKT, QK_M * D], F8, tag="wql", name="wql")
        wvh_s = pers.tile([D, NKT, D], F8, tag="wvh", name="wvh")
        wvl_s = pers.tile([D, NKT, D], F8, tag="wvl", name="wvl")
        woh_s = pers.tile([D, HPC, HID], F8, tag="woh", name="woh")
        wol_s = pers.tile([D, HPC, HID], F8, tag="wol", name="wol")
        cs_s = pers.tile([D, S], BF16, tag="cs", name="cs")
        sn_s = pers.tile([D, S], BF16, tag="sn", name="sn")
        mask_s = pers.tile([D, 4 * 512], BF16, tag="mask", name="mask")
        kT = pers.tile([D, S], BF16, tag="kT", name="kT")
        qT = [pers.tile([D, S], BF16, tag=f"qT{h}", name=f"qT{h}") for h in range(HPC)]
        # v natural layout + ones columns (=WQ_SCALE so the v quant scale
        # cancels against the denominator): per kt [64 d | one | 64 d | one]
        v_nat = pers.tile([D, NKT, 130], BF16, tag="vnat", name="vnat")
        a_hi = pers.tile([D, HPC, S], F8, tag="ahi", name="ahi")
        a_lo = pers.tile([D, HPC, S], F8, tag="alo", name="alo")
        ebias = pers.tile([D, 1], F32, tag="ebias", name="ebias")

        nc.vector.memset(ebias[:], -2.0)
        nc.vector.memset(v_nat[:, :, 64:65], WQ_SCALE)
        nc.vector.memset(v_nat[:, :, 129:130], WQ_SCALE)

        # input DMAs: hi planes (first compute pass) in kt-pair granularity
        # so matmuls can start as soon as pair 0 lands.
        for j in range(NKT // 2):
            js = slice(2 * j, 2 * j + 2)
            nc.sync.dma_start(xh_s[:, js, :], xh_d[:, js, :])
            nc.sync.dma_start(wqh_s[:, js, :], wqh_d[:, js, :])
        nc.sync.dma_start(wvh_s[:], wvh_d)
        nc.sync.dma_start(cs_s[:], cs_d)
        nc.sync.dma_start(sn_s[:], sn_d)
        for j in range(NKT // 2):
            js = slice(2 * j, 2 * j + 2)
            nc.sync.dma_start(xl_s[:, js, :], xl_d[:, js, :])
            nc.sync.dma_start(wql_s[:, js, :], wql_d[:, js, :])
        nc.sync.dma_start(wvl_s[:], wvl_d)
        nc.sync.dma_start(mask_s[:], mask_d)
        nc.sync.dma_start(woh_s[:], woh_d)
        nc.sync.dma_start(wol_s[:], wol_d)

        # ---------------- Stage A: QKV projection + RoPE + V ----------------
        with ExitStack() as sA:
            pA = sA.enter_context(tc.tile_pool(name="pA", bufs=2, space="PSUM"))
            rp = sA.enter_context(tc.tile_pool(name="rp", bufs=2))

            qk_passes = [(wqh_s, xh_s), (wql_s, xh_s), (wqh_s, xl_s)]
            v_passes = [(xh_s, wvh_s), (xh_s, wvl_s), (xl_s, wvh_s)]

            for ch in range(NCH):
                chsl = slice(ch * 512, (ch + 1) * 512)
                for m in range(QK_M):
                    qp = pA.tile([D, 512], F32, tag="qp", name="qp")
                    n = 0
                    for wt, xt in qk_passes:
                        for j in range(NKT // 2):
                            js = slice(2 * j, 2 * j + 2)
                            nc.tensor.matmul(
                                qp[:], wt[:, js, m * D:(m + 1) * D], xt[:, js, chsl],
                                start=(n == 0), stop=(n == 23), perf_mode=DR)
                            n += 1
                    # rope: out = x*cs2 + swap(x*sn2), sn2 = [sin; -sin] so
                    # the half-swap lands the signed cross terms correctly.
                    e_t = rp.tile([D, 512], BF16, tag="e", name="e")
                    nc.vector.tensor_copy(e_t[:], qp[:])
                    ra = rp.tile([D, 512], BF16, tag="ra", name="ra")
                    rb = rp.tile([D, 512], BF16, tag="rb", name="rb")
                    nc.vector.tensor_tensor(ra[:], e_t[:], cs_s[:, chsl], ALU.mult)
                    nc.vector.tensor_tensor(rb[:], e_t[:], sn_s[:, chsl], ALU.mult)
                    rbs = rp.tile([D, 512], BF16, tag="rbs", name="rbs")
                    nc.sync.dma_start(rbs[0:64, :], rb[64:128, :])
                    nc.sync.dma_start(rbs[64:128, :], rb[0:64, :])
                    dest = qT[m] if m < HPC else kT
                    nc.vector.tensor_tensor(dest[0:64, chsl], ra[0:64, :],
                                            rbs[0:64, :], ALU.add)
                    nc.vector.tensor_tensor(dest[64:128, chsl], ra[64:128, :],
                                            rbs[64:128, :], ALU.add)
                for t in range(4 * ch, 4 * ch + 4):
                    vp = pA.tile([D, D], F32, tag="vp", name="vp")
                    n = 0
                    for xt, wt in v_passes:
                        for j in range(NKT // 2):
                            js = slice(2 * j, 2 * j + 2)
                            nc.tensor.matmul(
                                vp[:], xt[:, js, t * D:(t + 1) * D], wt[:, js, :],
                                start=(n == 0), stop=(n == 23), perf_mode=DR)
                            n += 1
                    nc.scalar.copy(v_nat[:, t, 0:64], vp[:, 0:64])
                    nc.scalar.copy(v_nat[:, t, 65:129], vp[:, 64:128])

        if stages < 2:
            for ncx in range(4):
                csl = slice(ncx * 512, (ncx + 1) * 512)
                for sdx, src_t in enumerate([kT, qT[0], qT[3]]):
                    nc.sync.dma_start(out_d[sdx * D:(sdx + 1) * D, csl], src_t[:, csl])
            for t in range(NKT):
                nc.sync.dma_start(out_d[3 * D:4 * D, t * 128:(t + 1) * 128],
                                  v_nat[:, t, 0:128])

        # ---------------- Stage B: attention + o_proj (qc-pipelined) --------
        if stages < 2:
            nc.compile()
            return nc
        pSC = octx.enter_context(tc.tile_pool(name="pSC", bufs=1, space="PSUM"))
        pPV = octx.enter_context(tc.tile_pool(name="pPV", bufs=2, space="PSUM"))
        pYP = octx.enter_context(tc.tile_pool(name="pYP", bufs=2, space="PSUM"))
        ptp = octx.enter_context(tc.tile_pool(name="ptp", bufs=2))
        nrm = octx.enter_context(tc.tile_pool(name="nrm", bufs=2))
        ybp = octx.enter_context(tc.tile_pool(name="ybp", bufs=4))

        def emit_pv(pv0, pv1, pt, kts, nblk):
            for i, kt in enumerate(kts):
                psl = slice(i * 512, (i + 1) * 512)
                nc.tensor.matmul(pv0[0:65, :], v_nat[:, kt, 0:65], pt[:, psl],
                                 start=(kt == 0), stop=(kt == nblk - 1))
                nc.tensor.matmul(pv1[0:65, :], v_nat[:, kt, 65:130], pt[:, psl],
                                 start=(kt == 0), stop=(kt == nblk - 1))

        def emit_oproj(qc):
            for t in range(4 * qc, 4 * qc + 4):
                tsl = slice(t * D, (t + 1) * D)
                for cc in range(4):
                    ccsl = slice(cc * 512, (cc + 1) * 512)
                    yp = pYP.tile([D, 512], F32, tag="yp", name="yp")
                    n = 0
                    for A, W in ((a_hi, woh_s), (a_hi, wol_s), (a_lo, woh_s)):
                        for j in range(2):
                            js = slice(2 * j, 2 * j + 2)
                            nc.tensor.matmul(
                                yp[:], A[:, js, tsl], W[:, js, ccsl],
                                start=(n == 0), stop=(n == 5), perf_mode=DR)
                            n += 1
                    yt = ybp.tile([D, 512], BF16, tag="yt", name="yt")
                    if (t + cc) % 2 == 0:
                        nc.vector.tensor_copy(yt[:], yp[:])
                    else:
                        nc.scalar.copy(yt[:], yp[:])
                    nc.sync.dma_start(out_d[tsl, ccsl], yt[:])

        for qc in range(NQC):
            nblk = 4 * qc + 4
            qsl = slice(qc * 512, (qc + 1) * 512)
            for h in range(HPC):
                pv0 = pPV.tile([D, 512], F32, tag="pv", name="pv0")
                pv1 = pPV.tile([D, 512], F32, tag="pv", name="pv1")
                prev = None
                for g0 in range(0, nblk, 4):
                    kts = list(range(g0, g0 + 4))
                    sc = pSC.tile([D, 2048], F32, tag="sc", name="sc")
                    for i, kt in enumerate(kts):
                        nc.tensor.matmul(
                            sc[:, i * 512:(i + 1) * 512],
                            kT[:, kt * D:(kt + 1) * D], qT[h][:, qsl],
                            start=True, stop=True)
                    pt = ptp.tile([D, 2048], BF16, tag="pt", name="pt")
                    nc.scalar.activation(pt[:], sc[:], AF.Exp, bias=ebias[:],
                                         scale=SCALE / (WQ_SCALE * WQ_SCALE))
                    for i, kt in enumerate(kts):
                        r = kt - 4 * qc
                        if r >= 0:
                            psl = slice(i * 512, (i + 1) * 512)
                            nc.vector.tensor_tensor(
                                pt[:, psl], pt[:, psl],
                                mask_s[:, r * 512:(r + 1) * 512], ALU.mult)
                    if prev is not None:
                        emit_pv(pv0, pv1, *prev, nblk)
                    prev = (pt, kts)
                emit_pv(pv0, pv1, *prev, nblk)
                # normalize + hi/lo split for o_proj
                rc = nrm.tile([1, 512], F32, tag="rc", name="rc")
                nc.vector.reciprocal(rc[:], pv0[64:65, :])
                rcb = nrm.tile([D, 512], F32, tag="rcb", name="rcb")
                nc.gpsimd.partition_broadcast(rcb[:], rc[:])
                t_bf = nrm.tile([D, 512], BF16, tag="tbf", name="tbf")
                nc.vector.tensor_tensor(t_bf[0:64, :], pv0[0:64, :],
                                        rcb[0:64, :], ALU.mult)
                nc.vector.tensor_tensor(t_bf[64:128, :], pv1[0:64, :],
                                        rcb[64:128, :], ALU.mult)
                nc.gpsimd.tensor_copy(a_hi[:, h, qsl], t_bf[:])
                nc.vector.tensor_tensor(a_lo[:, h, qsl], t_bf[:],
                                        a_hi[:, h, qsl], ALU.subtract)
            if qc > 0:
                emit_oproj(qc - 1)
        emit_oproj(NQC - 1)

    nc.compile()
    return nc


def _get_program():
    global _prog
    if _prog is None:
        _prog = _build_program()
    return _prog


def _f8(x):
    import ml_dtypes
    return np.ascontiguousarray(x).astype(ml_dtypes.float8_e4m3)


def _hl(x, scale):
    """Two-plane e4m3 split of x*scale (hi + lo ~= x*scale to ~0.1%)."""
    import ml_dtypes
    xs = (x * scale).astype(np.float32)
    hi = xs.astype(ml_dtypes.float8_e4m3)
    lo = (xs - hi.astype(np.float32)).astype(ml_dtypes.float8_e4m3)
    return np.ascontiguousarray(hi), np.ascontiguousarray(lo)


def _host_tables(positions_b):
    import ml_dtypes
    inv_freq = (1.0 / (ROPE_THETA ** (np.arange(0, D, 2, dtype=np.float32) / D)))
    ang = positions_b.astype(np.float32)[:, None] * inv_freq[None, :].astype(np.float32)
    cosT = np.cos(ang).T.astype(np.float32)
    sinT = np.sin(ang).T.astype(np.float32)
    cs2 = np.concatenate([cosT, cosT], axis=0)
    sn2 = np.concatenate([sinT, -sinT], axis=0)
    return (np.ascontiguousarray(cs2.astype(ml_dtypes.bfloat16)),
            np.ascontiguousarray(sn2.astype(ml_dtypes.bfloat16)))


def _host_masks():
    import ml_dtypes
    k = np.arange(D)[:, None]
    j = np.arange(512)[None, :]
    pats = [((m * D + k) <= j).astype(np.float32) for m in range(4)]
    masks = np.concatenate(pats, axis=1)
    return np.ascontiguousarray(masks.astype(ml_dtypes.bfloat16))


def kernel(positions, hidden_states, w_qkv, w_o):
    from concourse.bass_utils import run_bass_kernel_spmd

    nc = _get_program()

    positions = np.asarray(positions)
    hidden_states = np.asarray(hidden_states, dtype=np.float32)
    w_qkv = np.asarray(w_qkv, dtype=np.float32)
    w_o = np.asarray(w_o, dtype=np.float32)

    masks = _host_masks()
    tables = [_host_tables(positions[b]) for b in range(B)]
    # X^T in [128, kt, tok] layout, split into e4m3 hi/lo planes (per batch)
    xhl = []
    for b in range(B):
        xt = hidden_states[b].T.reshape(NKT, D, S).transpose(1, 0, 2)
        xhl.append(_hl(xt, 1.0))

    in_maps = []
    for c in range(2 * TP):
        b, g = c // TP, c % TP
        wq_cols = np.concatenate([
            w_qkv[:, g * HPC * D:(g + 1) * HPC * D],           # 4 q heads
            w_qkv[:, Q_SIZE + g * D: Q_SIZE + (g + 1) * D],    # k head g
        ], axis=1)                                             # [2048, 640]
        wv_col = w_qkv[:, Q_SIZE + KV_SIZE + g * D: Q_SIZE + KV_SIZE + (g + 1) * D]
        wqh, wql = _hl(wq_cols.reshape(NKT, D, QK_M * D).transpose(1, 0, 2), WQ_SCALE)
        wvh, wvl = _hl(wv_col.reshape(NKT, D, D).transpose(1, 0, 2), WQ_SCALE)
        wo_sl = w_o[g * HPC * D:(g + 1) * HPC * D, :]          # [512, 2048]
        woh, wol = _hl(wo_sl.reshape(HPC, D, HID).transpose(1, 0, 2), WO_SCALE)
        cs2, sn2 = tables[b]
        xh, xl = xhl[b]
        in_maps.append({
            "xh": xh, "xl": xl,
            "wqh": wqh, "wql": wql, "wvh": wvh, "wvl": wvl,
            "woh": woh, "wol": wol,
            "cs2": cs2, "sn2": sn2, "masks": masks,
        })

    res = run_bass_kernel_spmd(nc, in_maps, core_ids=list(range(2 * TP)))

    out = np.zeros((B, S, HID), dtype=np.float32)
    for c in range(2 * TP):
        b = c // TP
        out[b] += res.results[c]["out"].astype(np.float32)
    out *= 1.0 / WO_SCALE
    return out
